# revision 1
# baseline (speedup 1.0000x reference)
"""Trainium2 Bass kernel for the NEUROPULS unitary NxN photonic mesh.

Reference math: accumulate arch = (chain of structured 256x256 complex
factors) starting from X = diag(exp(i*theta_0)):
  for it in 1..127:  X <- CR @ MMI @ diag(p_it) @ MMI @ X
  it=128:            X <- MMI @ diag(p_128) @ MMI @ X
  final:             X <- diag(p_129) @ X
MMI is block-diagonal 2x2 over even pairs (2k,2k+1); CR is block-diagonal 2x2
over odd pairs (2k+1,2k+2) with passthrough rows 0,255.

Key identity: E_it := MMI @ diag(p) @ MMI is again block-2x2 over even pairs:
  out[2k+e] = d1[2k+e]*X[2k+e] + d2[2k]*X[2k+(1-e)]
  d1 = at^2*p - ar^2*p^sigma_e,  d2 = i*at*ar*(p + p^sigma_e)  (pair-symmetric)
CR step: out = g1 .* X + g2 .* S_o(X) with g1 = acr*sqrt(CT) on mid rows and
acr*sqrt(1-CT) on rows 0/255; g2 = i*acr*sqrt(1-CT) mid, 0 at rows 0/255.
The row-0/255 passthrough is folded into E's coefficients (pre-scaled by
thru/g1s) so CR uses uniform immediate scalars; the odd-pair partner shift is
done on the TensorEngine with sub/super-diagonal permutation matrices whose
zero edge rows implement the g2 edge zeros for free.

Layout per core (column shard of 32): partition k = even-pair index (0..127),
free = (e in 2, plane in {R=0,I=1}, c in 32) -> one [128,2,2,32] fp32 tile.
"""

import numpy as np

import concourse.bass as bass
import concourse.mybir as mybir
import concourse.tile as tile
from concourse.bass_utils import run_bass_kernel_spmd

N = 256
NCORES = 8
CPC = N // NCORES  # columns per core = 32
NITS = N // 2      # 128 E-steps; CR after the first 127

IL_MMI = 0.02
IMB = 0.01
IL_CR = 0.02
CT = 0.01

_A_MMI = float(np.sqrt(1.0 - IL_MMI))
AT = _A_MMI * float(np.sqrt((1.0 + IMB) / 2.0))  # MMI diag amplitude
AR = _A_MMI * float(np.sqrt((1.0 - IMB) / 2.0))  # MMI off-diag amplitude (x i)
_A_CR = float(np.sqrt(1.0 - IL_CR))
G1S = _A_CR * float(np.sqrt(CT))        # CR diag (mid rows)
G2C = _A_CR * float(np.sqrt(1.0 - CT))  # CR off-diag (x i); also thru
EDGE = G2C / G1S                        # pre-scale for rows 0/255 of E coeffs

F32 = mybir.dt.float32
I32 = mybir.dt.int32
MULT = mybir.AluOpType.mult
ADD = mybir.AluOpType.add
ISEQ = mybir.AluOpType.is_equal
SIN = mybir.ActivationFunctionType.Sin
PI = float(np.pi)


# Engine -> own-semaphore name prefix. Same-engine semaphore waits are
# redundant on strict-FIFO engines (hardware DRAIN enforces output hazards),
# and this walrus build rejects instructions with >1 sync wait, so we strip
# them after Tile scheduling.
_ENGINE_SEM_PREFIXES = {
    "DVE": ("DVE_",),
    "ACT": ("ACT_", "Activation_"),
    "PE": ("PE_",),
    "POOL": ("Pool_", "POOL_"),
    "SP": ("SP_",),
}


def strip_same_engine_waits(nc, verbose=False):
    multi = []
    for bb in nc.main_func.blocks:
        for ins in bb.instructions:
            si = getattr(ins, "sync_info", None)
            if si is None:
                continue
            eng = getattr(ins, "engine", None)
            pres = _ENGINE_SEM_PREFIXES.get(getattr(eng, "name", ""), ())
            if not pres:
                continue
            kept = [
                w
                for w in si.on_wait
                if not (
                    w.sync_type == "semaphore"
                    and w.ant_name
                    and w.ant_name.startswith(pres)
                )
            ]
            if len(kept) != len(si.on_wait):
                si.on_wait = kept
                ins.sync_info = si
            if len(kept) > 1:
                multi.append((ins.name, type(ins).__name__, [w.ant_name for w in kept]))
    if verbose and multi:
        print(f"[strip_waits] {len(multi)} instructions still multi-wait:")
        for m in multi[:20]:
            print("   ", m)
    return multi


def split_multi_waits(nc):
    """This walrus build allows one sync-wait per instruction: hoist extra
    waits onto same-engine Drain nops inserted just before the instruction."""
    n_split = 0
    for bb in nc.main_func.blocks:
        insts = bb.instructions
        i = 0
        while i < len(insts):
            ins = insts[i]
            si = getattr(ins, "sync_info", None)
            if si is None or len(si.on_wait) <= 1:
                i += 1
                continue
            waits = list(si.on_wait)
            for k, w in enumerate(waits[:-1]):
                d = mybir.InstDrain(
                    name=f"{ins.name}_waitsplit{k}", ins=[], outs=[]
                )
                d.engine = ins.engine
                import bass_rust as _br

                d.sync_info = _br.SyncInfo(on_wait=[w], on_update=[])
                insts.insert(i, d)
                i += 1
                n_split += 1
            si.on_wait = [waits[-1]]
            ins.sync_info = si
            i += 1
    return n_split


def fix_sync_waits(nc):
    strip_same_engine_waits(nc)
    return split_multi_waits(nc)


def build_nc(nits=NITS, with_final=True, repeat=1, tg_act=False, t0_act=False, e1_first=False):
    nc = bass.Bass()

    thetas = nc.dram_tensor("thetas", [130, N], F32, kind="ExternalInput")
    mask0 = nc.dram_tensor("mask0", [128, 2, 2, CPC], F32, kind="ExternalInput")
    # constant index masks: 4 shift-permutation weights (lhsT form) and the
    # CR diag vectors with passthrough edges
    wconst = nc.dram_tensor("wconst", [4, 128, 128], F32, kind="ExternalInput")
    gconst = nc.dram_tensor("gconst", [128, 2], F32, kind="ExternalInput")
    out_d = nc.dram_tensor("out", [128, 2, 2, CPC], F32, kind="ExternalOutput")

    with tile.TileContext(nc) as tc:
        with (
            tc.tile_pool(name="state", bufs=1) as sp,
            tc.tile_pool(name="coef", bufs=1) as cp,
            tc.tile_pool(name="psum", bufs=2, space="PSUM") as pp,
        ):
            # ------------- setup: trig + structured-step coefficients -------------
            th = cp.tile([128, 130, 2], F32, tag="th")   # theta[k,(it,e)]
            Ct = cp.tile([128, 130, 2], F32, tag="Ct")   # cos
            St = cp.tile([128, 130, 2], F32, tag="St")   # sin
            wrk = cp.tile([128, 130, 2], F32, tag="wrk")
            d1r = cp.tile([128, NITS, 2], F32, tag="d1r")   # index j = it-1
            d1i = cp.tile([128, NITS, 2], F32, tag="d1i")
            d1iN = cp.tile([128, NITS, 2], F32, tag="d1iN")
            d2r = cp.tile([128, NITS, 2], F32, tag="d2r")
            d2i = cp.tile([128, NITS, 2], F32, tag="d2i")
            d2iN = cp.tile([128, NITS, 2], F32, tag="d2iN")
            zb = cp.tile([128, 1], F32, tag="zb")
            Wt = cp.tile([128, 4, 128], F32, tag="Wt")
            gv = cp.tile([128, 2], F32, tag="gv")
            m0 = cp.tile([128, 2, 2, CPC], F32, tag="m0")
            sN = cp.tile([128, 2], F32, tag="sN")  # -sin(theta_129)

            nc.sync.dma_start(
                th[:], thetas[:].rearrange("it (k e) -> k it e", k=128, e=2)
            )
            nc.sync.dma_start(m0[:], mask0[:])
            nc.sync.dma_start(gv[:], gconst[:])
            nc.sync.dma_start(Wt[:], wconst[:].rearrange("w p f -> p w f"))
            nc.vector.memset(zb[:], 0.0)

            # sin/cos with range reduction into (-pi, pi]:
            #   v = th (+ pi/2 for cos); v -= 2*pi if v > pi
            wrp = cp.tile([128, 130, 2], F32, tag="wrp")
            nc.vector.tensor_scalar(wrp[:], th[:], PI, -2 * PI, mybir.AluOpType.is_gt, MULT)
            nc.vector.tensor_tensor(wrk[:], th[:], wrp[:], ADD)
            nc.scalar.activation(St[:], wrk[:], SIN, bias=zb[:])
            nc.vector.tensor_scalar(wrk[:], th[:], PI / 2, None, ADD)
            nc.vector.tensor_scalar(wrp[:], wrk[:], PI, -2 * PI, mybir.AluOpType.is_gt, MULT)
            nc.vector.tensor_tensor(wrk[:], wrk[:], wrp[:], ADD)
            nc.scalar.activation(Ct[:], wrk[:], SIN, bias=zb[:])

            # layer views it = 1..128 and their e-swapped counterparts
            Cmid = Ct[:, 1 : NITS + 1, :]
            Smid = St[:, 1 : NITS + 1, :]
            Csw = Ct[:, 1 : NITS + 1, ::-1]
            Ssw = St[:, 1 : NITS + 1, ::-1]
            wmid = wrk[:, :NITS, :]

            # d1 = at^2 p - ar^2 p^sigma ; d2 = i at ar (p + p^sigma)
            nc.vector.tensor_scalar(wmid, Csw, -AR * AR, None, MULT)
            nc.vector.scalar_tensor_tensor(d1r[:], Cmid, AT * AT, wmid, MULT, ADD)
            nc.vector.tensor_scalar(wmid, Ssw, -AR * AR, None, MULT)
            nc.vector.scalar_tensor_tensor(d1i[:], Smid, AT * AT, wmid, MULT, ADD)
            nc.vector.tensor_tensor(wmid, Smid, Ssw, ADD)
            nc.vector.tensor_scalar(d2r[:], wmid, -AT * AR, None, MULT)
            nc.vector.tensor_tensor(wmid, Cmid, Csw, ADD)
            nc.vector.tensor_scalar(d2i[:], wmid, AT * AR, None, MULT)

            nc.vector.tensor_scalar(d1iN[:], d1i[:], -1.0, None, MULT)
            nc.vector.tensor_scalar(d2iN[:], d2i[:], -1.0, None, MULT)
            nc.vector.tensor_scalar(sN[:], St[:, NITS + 1, :], -1.0, None, MULT)

            # host-supplied constants: shift weights + CR diag vectors
            Wdn = Wt[:, 0, :]
            WdnN = Wt[:, 1, :]
            Wup = Wt[:, 2, :]
            WupN = Wt[:, 3, :]
            esc0 = gv[:, 0:1]
            esc1 = gv[:, 1:2]

            # ------------- state init: X = diag(p_0) -------------
            X = sp.tile([128, 2, 2, CPC], F32, tag="X")
            Y = sp.tile([128, 2, 2, CPC], F32, tag="Y")
            u = sp.tile([128, 2, 2, CPC], F32, tag="u")
            t0 = sp.tile([128, 2, CPC], F32, tag="t0")
            t1 = sp.tile([128, 2, CPC], F32, tag="t1")
            tg = sp.tile([128, 2, 2, CPC], F32, tag="tg")

            for e in range(2):
                c0 = Ct[:, 0, e : e + 1]
                s0 = St[:, 0, e : e + 1]
                nc.vector.tensor_scalar(X[:, e, 0, :], m0[:, e, 0, :], c0, None, MULT)
                nc.vector.tensor_scalar(X[:, e, 1, :], m0[:, e, 1, :], s0, None, MULT)

            # ------------- main chain -------------
            for _rep in range(repeat):
              for it in range(1, nits + 1):
                j = it - 1
                cd1r = [d1r[:, j, e : e + 1] for e in range(2)]
                cd1i = [d1i[:, j, e : e + 1] for e in range(2)]
                cd1iN = [d1iN[:, j, e : e + 1] for e in range(2)]
                cd2r = d2r[:, j, 0:1]
                cd2i = d2i[:, j, 0:1]
                cd2iN = d2iN[:, j, 0:1]

                # --- E-step: Y = E_it(X) ---
                # d2 part over both e at once (e-swapped reads):
                #   u[:,:,0,:] = d2r*XswR - d2i*XswI   (R contribution)
                #   u[:,:,1,:] = d2i*XswR + d2r*XswI   (I contribution)
                XswR = X[:, ::-1, 0, :]
                XswI = X[:, ::-1, 1, :]
                if t0_act:
                    nc.scalar.mul(t0[:], XswI, cd2iN)
                    nc.scalar.mul(t1[:], XswI, cd2r)
                else:
                    nc.vector.tensor_scalar(t0[:], XswI, cd2iN, None, MULT)
                    nc.vector.tensor_scalar(t1[:], XswI, cd2r, None, MULT)
                nc.vector.scalar_tensor_tensor(
                    u[:, :, 0, :], XswR, cd2r, t0[:], MULT, ADD
                )
                nc.vector.scalar_tensor_tensor(
                    u[:, :, 1, :], XswR, cd2i, t1[:], MULT, ADD
                )
                # d1 part per e (chained through Y slices):
                e_order = (1, 0) if e1_first else (0, 1)
                for e in e_order:
                    nc.vector.scalar_tensor_tensor(
                        Y[:, e, 0, :], X[:, e, 1, :], cd1iN[e], u[:, e, 0, :], MULT, ADD
                    )
                    nc.vector.scalar_tensor_tensor(
                        Y[:, e, 0, :], X[:, e, 0, :], cd1r[e], Y[:, e, 0, :], MULT, ADD
                    )
                    nc.vector.scalar_tensor_tensor(
                        Y[:, e, 1, :], X[:, e, 0, :], cd1i[e], u[:, e, 1, :], MULT, ADD
                    )
                    nc.vector.scalar_tensor_tensor(
                        Y[:, e, 1, :], X[:, e, 1, :], cd1r[e], Y[:, e, 1, :], MULT, ADD
                    )

                if it == nits:
                    # last iteration: no crossing
                    break

                # --- O-step (CR): X = g1s*Y + g2 .* S_o(Y) ---
                # PE computes sgP[:,e,0,:] = -S_o(Y_I)(e), sgP[:,e,1,:] = +S_o(Y_R)(e)
                sgP = pp.tile([128, 2, 2, CPC], F32, tag="sgP")
                nc.tensor.matmul(sgP[:, 0, 0, :], WdnN, Y[:, 1, 1, :], start=True, stop=True)
                nc.tensor.matmul(sgP[:, 0, 1, :], Wdn, Y[:, 1, 0, :], start=True, stop=True)
                nc.tensor.matmul(sgP[:, 1, 0, :], WupN, Y[:, 0, 1, :], start=True, stop=True)
                nc.tensor.matmul(sgP[:, 1, 1, :], Wup, Y[:, 0, 0, :], start=True, stop=True)
                # CR diag term
                if tg_act:
                    nc.scalar.mul(tg[:, 0, :, :], Y[:, 0, :, :], esc0)
                    nc.scalar.mul(tg[:, 1, :, :], Y[:, 1, :, :], esc1)
                else:
                    nc.vector.tensor_scalar(tg[:, 0, :, :], Y[:, 0, :, :], esc0, None, MULT)
                    nc.vector.tensor_scalar(tg[:, 1, :, :], Y[:, 1, :, :], esc1, None, MULT)
                nc.vector.scalar_tensor_tensor(X[:], sgP[:], G2C, tg[:], MULT, ADD)

            if with_final:
                # ------------- final: X = diag(p_129) @ Y -------------
                for e in range(2):
                    c129 = Ct[:, NITS + 1, e : e + 1]
                    s129 = St[:, NITS + 1, e : e + 1]
                    s129N = sN[:, e : e + 1]
                    nc.vector.tensor_scalar(t0[:, e, :], Y[:, e, 1, :], s129N, None, MULT)
                    nc.vector.scalar_tensor_tensor(
                        X[:, e, 0, :], Y[:, e, 0, :], c129, t0[:, e, :], MULT, ADD
                    )
                    nc.vector.tensor_scalar(t0[:, e, :], Y[:, e, 0, :], s129, None, MULT)
                    nc.vector.scalar_tensor_tensor(
                        X[:, e, 1, :], Y[:, e, 1, :], c129, t0[:, e, :], MULT, ADD
                    )
                nc.sync.dma_start(out_d[:], X[:])
            else:
                nc.sync.dma_start(out_d[:], Y[:])

    return nc


def make_consts():
    """Constant index masks: shift-permutation lhsT weights + CR diag vectors."""
    wdn = np.eye(128, k=1, dtype=np.float32)   # lhsT[p,f] = (f == p+1)
    wup = np.eye(128, k=-1, dtype=np.float32)  # lhsT[p,f] = (f == p-1)
    wconst = np.stack([wdn, -wdn, wup, -wup]).astype(np.float32)
    g = np.full((128, 2), G1S, dtype=np.float32)
    g[0, 0] = G2C
    g[127, 1] = G2C
    return wconst, g


def make_mask0(core: int) -> np.ndarray:
    """mask0[k,e,l,c] = 1 iff global row 2k+e == global col 32*core+c."""
    k = np.arange(128)[:, None, None, None]
    e = np.arange(2)[None, :, None, None]
    c = np.arange(CPC)[None, None, None, :]
    m = (2 * k + e == CPC * core + c).astype(np.float32)
    return np.broadcast_to(m, (128, 2, 2, CPC)).copy()


_CACHE = {}


def _get_nc():
    if "nc" not in _CACHE:
        nc = build_nc()
        fix_sync_waits(nc)
        _CACHE["nc"] = nc
    return _CACHE["nc"]


def _run(thetas: np.ndarray, trace: bool = False):
    thetas = np.ascontiguousarray(thetas, dtype=np.float32)
    assert thetas.shape == (130, N)
    nc = _get_nc()
    wconst, gconst = make_consts()
    in_maps = [
        {"thetas": thetas, "mask0": make_mask0(c), "wconst": wconst, "gconst": gconst}
        for c in range(NCORES)
    ]
    res = run_bass_kernel_spmd(nc, in_maps, list(range(NCORES)), trace=trace)
    out = np.empty((N, N), dtype=np.complex64)
    for c in range(NCORES):
        o = res.results[c]["out"]  # [128, 2, 2, CPC]
        blk = o[:, :, 0, :] + 1j * o[:, :, 1, :]  # [128, 2, CPC]
        out[:, CPC * c : CPC * (c + 1)] = blk.reshape(N, CPC)
    return out, res


def kernel(thetas: np.ndarray) -> np.ndarray:
    out, _ = _run(thetas, trace=False)
    return out



# revision 30
# speedup vs baseline: 1.4431x; 1.4431x over previous
"""Trainium2 Bass kernel for the NEUROPULS unitary NxN photonic mesh.

Reference math: accumulate arch = (chain of structured 256x256 complex
factors) starting from X = diag(exp(i*theta_0)):
  for it in 1..127:  X <- CR @ MMI @ diag(p_it) @ MMI @ X
  it=128:            X <- MMI @ diag(p_128) @ MMI @ X
  final:             X <- diag(p_129) @ X
MMI is block-diagonal 2x2 over even pairs (2k,2k+1); CR is block-diagonal 2x2
over odd pairs (2k+1,2k+2) with passthrough rows 0,255.

Key identity: E_it := MMI @ diag(p) @ MMI is again block-2x2 over even pairs:
  out[2k+e] = d1[2k+e]*X[2k+e] + d2[2k]*X[2k+(1-e)]
  d1 = at^2*p - ar^2*p^sigma_e,  d2 = i*at*ar*(p + p^sigma_e)  (pair-symmetric)
CR step: out = g1 .* Y + g2 .* S_o(Y); the whole CR step runs on the
TensorEngine as PSUM-accumulated matmuls: shift-permutation weights
(pre-scaled by g2) plus diag(g1) weights, so PSUM holds the next state X
directly.  The row-0/255 passthrough lives in the diag(g1) weights.

Engine split per iteration (cost-model driven):
  ACT : t0           (per-partition scalar multiplies)
  Pool: t1, u0, u1, one Y chain, PSUM->SBUF state copies
  DVE : three Y chains
  PE  : 8 matmuls (4 shift + 4 diag) accumulating the CR result in PSUM

Layout per core (column shard of 32): partition k = even-pair index (0..127),
free = (e in 2, plane in {R=0,I=1}, c in 32) -> one [128,2,2,32] fp32 tile.
"""

import numpy as np

import concourse.bass as bass
import concourse.mybir as mybir
import concourse.tile as tile
from concourse.bass_utils import run_bass_kernel_spmd

N = 256
NCORES = 8
CPC = N // NCORES  # columns per core = 32
NITS = N // 2      # 128 E-steps; CR after the first 127

IL_MMI = 0.02
IMB = 0.01
IL_CR = 0.02
CT = 0.01

_A_MMI = float(np.sqrt(1.0 - IL_MMI))
AT = _A_MMI * float(np.sqrt((1.0 + IMB) / 2.0))  # MMI diag amplitude
AR = _A_MMI * float(np.sqrt((1.0 - IMB) / 2.0))  # MMI off-diag amplitude (x i)
_A_CR = float(np.sqrt(1.0 - IL_CR))
G1S = _A_CR * float(np.sqrt(CT))        # CR diag (mid rows)
G2C = _A_CR * float(np.sqrt(1.0 - CT))  # CR off-diag (x i); also thru

F32 = mybir.dt.float32
MULT = mybir.AluOpType.mult
ADD = mybir.AluOpType.add
SIN = mybir.ActivationFunctionType.Sin
COPY = mybir.ActivationFunctionType.Copy
PI = float(np.pi)

_ENG = {
    "dve": lambda nc: nc.vector,
    "pool": lambda nc: nc.gpsimd,
}


# Engine -> own-semaphore name prefix. Same-engine semaphore waits are
# redundant on strict-FIFO engines (hardware DRAIN enforces output hazards),
# and this walrus build rejects instructions with >1 sync wait, so we strip
# them after Tile scheduling.
# NOTE: Pool (gpsimd, parallel Q7 DSP cores) is intentionally absent — its
# instructions are not guaranteed to retire in FIFO order, so same-engine
# semaphore waits there are load-bearing and must be kept (Drain-split when
# an instruction carries more than one wait).
_ENGINE_SEM_PREFIXES = {
    "DVE": ("DVE_",),
    "ACT": ("ACT_", "Activation_"),
    "PE": ("PE_",),
    "SP": ("SP_",),
}


def strip_same_engine_waits(nc, verbose=False):
    multi = []
    for bb in nc.main_func.blocks:
        for ins in bb.instructions:
            si = getattr(ins, "sync_info", None)
            if si is None:
                continue
            eng = getattr(ins, "engine", None)
            pres = _ENGINE_SEM_PREFIXES.get(getattr(eng, "name", ""), ())
            if not pres:
                continue
            kept = [
                w
                for w in si.on_wait
                if not (
                    w.sync_type == "semaphore"
                    and w.ant_name
                    and w.ant_name.startswith(pres)
                )
            ]
            if len(kept) != len(si.on_wait):
                si.on_wait = kept
                ins.sync_info = si
            if len(kept) > 1:
                multi.append((ins.name, type(ins).__name__, [w.ant_name for w in kept]))
    if verbose and multi:
        print(f"[strip_waits] {len(multi)} instructions still multi-wait:")
        for m in multi[:20]:
            print("   ", m)
    return multi


def split_multi_waits(nc):
    """This walrus build allows one sync-wait per instruction: hoist extra
    waits onto same-engine Drain nops inserted just before the instruction."""
    n_split = 0
    for bb in nc.main_func.blocks:
        insts = bb.instructions
        i = 0
        while i < len(insts):
            ins = insts[i]
            si = getattr(ins, "sync_info", None)
            if si is None or len(si.on_wait) <= 1:
                i += 1
                continue
            waits = list(si.on_wait)
            for k, w in enumerate(waits[:-1]):
                d = mybir.InstDrain(
                    name=f"{ins.name}_waitsplit{k}", ins=[], outs=[]
                )
                d.engine = ins.engine
                import bass_rust as _br

                d.sync_info = _br.SyncInfo(on_wait=[w], on_update=[])
                insts.insert(i, d)
                i += 1
                n_split += 1
            si.on_wait = [waits[-1]]
            ins.sync_info = si
            i += 1
    return n_split


def fix_sync_waits(nc):
    strip_same_engine_waits(nc)
    return split_multi_waits(nc)


def build_nc(
    nits=NITS,
    with_final=True,
    plan=None,
):
    """plan knobs (defaults tuned via TimelineSim):
    copy_order: sequence of ("dve"|"act", plane) PSUM->SBUF copies
    t0_psum/t1_psum: read t-inputs straight from PSUM
    t1_eng, u0_eng, u1_eng: dve|pool
    chains: emission order [(e, pl, eng), ...]
    """
    p = dict(
        copy_order=(("dve", 1), ("dve", 0)),
        t0_psum=True,
        t1_eng="dve", t1_psum=False,
        u0_eng="pool", u1_eng="dve",
        chains=((1, 0, "dve"), (0, 1, "pool"), (0, 0, "dve"), (1, 1, "pool")),
        mm_order=((0, 1), (1, 1), (0, 0), (1, 0)),
    )
    if plan:
        p.update(plan)
    nc = bass.Bass()

    thetas = nc.dram_tensor("thetas", [130, N], F32, kind="ExternalInput")
    mask0 = nc.dram_tensor("mask0", [128, 2, 2, CPC], F32, kind="ExternalInput")
    # constant weights: 4 shift-permutations (pre-scaled by g2) and the two
    # diag(g1) matrices, all in lhsT form
    wconst = nc.dram_tensor("wconst", [6, 128, 128], F32, kind="ExternalInput")
    out_d = nc.dram_tensor("out", [128, 2, 2, CPC], F32, kind="ExternalOutput")

    with tile.TileContext(nc) as tc:
        with (
            tc.tile_pool(name="state", bufs=1) as sp,
            tc.tile_pool(name="coef", bufs=1) as cp,
            tc.tile_pool(name="psum", bufs=2, space="PSUM") as pp,
        ):
            # ------------- setup: trig + structured-step coefficients -------------
            th = cp.tile([128, 130, 2], F32, tag="th")   # theta[k,(it,e)]
            Ct = cp.tile([128, 130, 2], F32, tag="Ct")   # cos
            St = cp.tile([128, 130, 2], F32, tag="St")   # sin
            wrk = cp.tile([128, 130, 2], F32, tag="wrk")
            d1r = cp.tile([128, NITS, 2], F32, tag="d1r")   # index j = it-1
            d1i = cp.tile([128, NITS, 2], F32, tag="d1i")
            d1iN = cp.tile([128, NITS, 2], F32, tag="d1iN")
            d2r = cp.tile([128, NITS, 2], F32, tag="d2r")
            d2i = cp.tile([128, NITS, 2], F32, tag="d2i")
            d2iN = cp.tile([128, NITS, 2], F32, tag="d2iN")
            zb = cp.tile([128, 1], F32, tag="zb")
            Wt = cp.tile([128, 6, 128], F32, tag="Wt")
            m0 = cp.tile([128, 2, 2, CPC], F32, tag="m0")
            sN = cp.tile([128, 2], F32, tag="sN")  # -sin(theta_129)

            nc.sync.dma_start(
                th[:], thetas[:].rearrange("it (k e) -> k it e", k=128, e=2)
            )
            nc.sync.dma_start(m0[:], mask0[:])
            nc.sync.dma_start(Wt[:], wconst[:].rearrange("w p f -> p w f"))
            nc.vector.memset(zb[:], 0.0)

            # sin/cos with range reduction into (-pi, pi]:
            #   v = th (+ pi/2 for cos); v -= 2*pi if v > pi
            wrp = cp.tile([128, 130, 2], F32, tag="wrp")
            nc.vector.tensor_scalar(wrp[:], th[:], PI, -2 * PI, mybir.AluOpType.is_gt, MULT)
            nc.vector.tensor_tensor(wrk[:], th[:], wrp[:], ADD)
            nc.scalar.activation(St[:], wrk[:], SIN, bias=zb[:])
            nc.vector.tensor_scalar(wrk[:], th[:], PI / 2, None, ADD)
            nc.vector.tensor_scalar(wrp[:], wrk[:], PI, -2 * PI, mybir.AluOpType.is_gt, MULT)
            nc.vector.tensor_tensor(wrk[:], wrk[:], wrp[:], ADD)
            nc.scalar.activation(Ct[:], wrk[:], SIN, bias=zb[:])

            # layer views it = 1..128 and their e-swapped counterparts
            Cmid = Ct[:, 1 : NITS + 1, :]
            Smid = St[:, 1 : NITS + 1, :]
            Csw = Ct[:, 1 : NITS + 1, ::-1]
            Ssw = St[:, 1 : NITS + 1, ::-1]
            wmid = wrk[:, :NITS, :]

            # d1 = at^2 p - ar^2 p^sigma ; d2 = i at ar (p + p^sigma)
            nc.vector.tensor_scalar(wmid, Csw, -AR * AR, None, MULT)
            nc.vector.scalar_tensor_tensor(d1r[:], Cmid, AT * AT, wmid, MULT, ADD)
            nc.vector.tensor_scalar(wmid, Ssw, -AR * AR, None, MULT)
            nc.vector.scalar_tensor_tensor(d1i[:], Smid, AT * AT, wmid, MULT, ADD)
            nc.vector.tensor_tensor(wmid, Smid, Ssw, ADD)
            nc.vector.tensor_scalar(d2r[:], wmid, -AT * AR, None, MULT)
            nc.vector.tensor_tensor(wmid, Cmid, Csw, ADD)
            nc.vector.tensor_scalar(d2i[:], wmid, AT * AR, None, MULT)

            nc.vector.tensor_scalar(d1iN[:], d1i[:], -1.0, None, MULT)
            nc.vector.tensor_scalar(d2iN[:], d2i[:], -1.0, None, MULT)
            nc.vector.tensor_scalar(sN[:], St[:, NITS + 1, :], -1.0, None, MULT)

            # host-supplied constant weights
            WdnG = Wt[:, 0, :]
            WdnNG = Wt[:, 1, :]
            WupG = Wt[:, 2, :]
            WupNG = Wt[:, 3, :]
            Desc0 = Wt[:, 4, :]
            Desc1 = Wt[:, 5, :]

            # ------------- state init: X = diag(p_0) -------------
            X = sp.tile([128, 2, 2, CPC], F32, tag="X")
            Y = sp.tile([128, 2, 2, CPC], F32, tag="Y")
            u = sp.tile([128, 2, 2, CPC], F32, tag="u")
            t0 = sp.tile([128, 2, CPC], F32, tag="t0")
            t1 = sp.tile([128, 2, CPC], F32, tag="t1")

            for e in range(2):
                c0 = Ct[:, 0, e : e + 1]
                s0 = St[:, 0, e : e + 1]
                nc.vector.tensor_scalar(X[:, e, 0, :], m0[:, e, 0, :], c0, None, MULT)
                nc.vector.tensor_scalar(X[:, e, 1, :], m0[:, e, 1, :], s0, None, MULT)

            # ------------- main chain -------------
            P = None
            for it in range(1, nits + 1):
                j = it - 1
                cd1r = [d1r[:, j, e : e + 1] for e in range(2)]
                cd1i = [d1i[:, j, e : e + 1] for e in range(2)]
                cd1iN = [d1iN[:, j, e : e + 1] for e in range(2)]
                cd2r = d2r[:, j, 0:1]
                cd2i = d2i[:, j, 0:1]
                cd2iN = d2iN[:, j, 0:1]

                # --- E-step: Y = E_it(X) ---
                # d2 part over both e at once (e-swapped reads):
                #   u[:,:,0,:] = d2r*XswR - d2i*XswI   (R contribution)
                #   u[:,:,1,:] = d2i*XswR + d2r*XswI   (I contribution)
                XswR = X[:, ::-1, 0, :]
                XswI = X[:, ::-1, 1, :]

                # t0: when reading PSUM it may be emitted before the copies;
                # when reading SBUF X it must come after the plane-1 copy.
                if P is not None and p["t0_psum"]:
                    nc.scalar.activation(t0[:], P[:, ::-1, 1, :], COPY, scale=cd2iN)

                if P is not None:
                    # materialize this iteration's input state from PSUM
                    # (GPSIMD cannot access PSUM; DVE/ACT only)
                    for ceng, cpl in p["copy_order"]:
                        if ceng == "dve":
                            nc.vector.tensor_copy(X[:, :, cpl, :], P[:, :, cpl, :])
                        elif ceng == "dve1":
                            nc.vector.tensor_copy(X[:], P[:])
                        else:
                            nc.scalar.activation(X[:, :, cpl, :], P[:, :, cpl, :], COPY)

                if not (P is not None and p["t0_psum"]):
                    if p.get("t0_eng", "act") == "act":
                        nc.scalar.activation(t0[:], XswI, COPY, scale=cd2iN)
                    else:
                        _ENG[p["t0_eng"]](nc).tensor_scalar(t0[:], XswI, cd2iN, None, MULT)

                t1_src = P[:, ::-1, 1, :] if (P is not None and p["t1_psum"]) else XswI
                _ENG[p["t1_eng"]](nc).tensor_scalar(t1[:], t1_src, cd2r, None, MULT)
                _ENG[p["u0_eng"]](nc).scalar_tensor_tensor(
                    u[:, :, 0, :], XswR, cd2r, t0[:], MULT, ADD
                )
                _ENG[p["u1_eng"]](nc).scalar_tensor_tensor(
                    u[:, :, 1, :], XswR, cd2i, t1[:], MULT, ADD
                )
                # d1 part per (e, plane): 2-op chains, emission order from plan
                for e, pl, yeng in p["chains"]:
                    eng = _ENG[yeng](nc)
                    xa = X[:, e, 1 - pl, :]
                    xb = X[:, e, pl, :]
                    ca = cd1iN[e] if pl == 0 else cd1i[e]
                    eng.scalar_tensor_tensor(
                        Y[:, e, pl, :], xa, ca, u[:, e, pl, :], MULT, ADD
                    )
                    eng.scalar_tensor_tensor(
                        Y[:, e, pl, :], xb, cd1r[e], Y[:, e, pl, :], MULT, ADD
                    )

                if it == nits:
                    # last iteration: no crossing
                    break

                # --- O-step (CR) fully on PE, accumulated in PSUM ---
                # P[:,e,pl,:] = g2*S_o(Y)-part + diag(g1)*Y[:,e,pl,:]
                # shift weights: P[:,0,0]<-WdnNG*Y[:,1,1]; P[:,0,1]<-WdnG*Y[:,1,0]
                #                P[:,1,0]<-WupNG*Y[:,0,1]; P[:,1,1]<-WupG*Y[:,0,0]
                shift_w = {(0, 0): WdnNG, (0, 1): WdnG, (1, 0): WupNG, (1, 1): WupG}
                desc_w = (Desc0, Desc1)
                P = pp.tile([128, 2, 2, CPC], F32, tag="P")
                for e, pl in p["mm_order"]:
                    nc.tensor.matmul(
                        P[:, e, pl, :], shift_w[(e, pl)], Y[:, 1 - e, 1 - pl, :],
                        start=True, stop=False,
                    )
                    nc.tensor.matmul(
                        P[:, e, pl, :], desc_w[e], Y[:, e, pl, :],
                        start=False, stop=True,
                    )

            if with_final:
                # ------------- final: X = diag(p_129) @ Y -------------
                for e in range(2):
                    c129 = Ct[:, NITS + 1, e : e + 1]
                    s129 = St[:, NITS + 1, e : e + 1]
                    s129N = sN[:, e : e + 1]
                    nc.vector.tensor_scalar(t0[:, e, :], Y[:, e, 1, :], s129N, None, MULT)
                    nc.vector.scalar_tensor_tensor(
                        X[:, e, 0, :], Y[:, e, 0, :], c129, t0[:, e, :], MULT, ADD
                    )
                    nc.vector.tensor_scalar(t0[:, e, :], Y[:, e, 0, :], s129, None, MULT)
                    nc.vector.scalar_tensor_tensor(
                        X[:, e, 1, :], Y[:, e, 1, :], c129, t0[:, e, :], MULT, ADD
                    )
                nc.sync.dma_start(out_d[:], X[:])
            else:
                nc.sync.dma_start(out_d[:], Y[:])

    return nc


def build_nc2(nits=NITS, plan=None):
    """Dual-chain variant: columns split 16+16 into two independent chains,
    phase-shifted by half an iteration so each chain's dependency stalls are
    filled by the other chain's ready work (engines are strict FIFO)."""
    p = dict(
        t0_eng="act",
        t1_eng="dve", u0_eng="pool", u1_eng="pool",
        chains=((1, 0, "dve"), (0, 1, "pool"), (0, 0, "dve"), (1, 1, "pool")),
        mm_order=((0, 1), (1, 1), (0, 0), (1, 0)),
    )
    if plan:
        p.update(plan)
    HC = CPC // 2  # 16 columns per chain

    nc = bass.Bass()
    thetas = nc.dram_tensor("thetas", [130, N], F32, kind="ExternalInput")
    mask0 = nc.dram_tensor("mask0", [128, 2, 2, CPC], F32, kind="ExternalInput")
    wconst = nc.dram_tensor("wconst", [6, 128, 128], F32, kind="ExternalInput")
    out_d = nc.dram_tensor("out", [128, 2, 2, CPC], F32, kind="ExternalOutput")

    with tile.TileContext(nc) as tc:
        with (
            tc.tile_pool(name="state", bufs=1) as sp,
            tc.tile_pool(name="coef", bufs=1) as cp,
            tc.tile_pool(name="psum", bufs=4, space="PSUM") as pp,
        ):
            # ---- setup (identical math to build_nc) ----
            th = cp.tile([128, 130, 2], F32, tag="th")
            Ct = cp.tile([128, 130, 2], F32, tag="Ct")
            St = cp.tile([128, 130, 2], F32, tag="St")
            wrk = cp.tile([128, 130, 2], F32, tag="wrk")
            d1r = cp.tile([128, NITS, 2], F32, tag="d1r")
            d1i = cp.tile([128, NITS, 2], F32, tag="d1i")
            d1iN = cp.tile([128, NITS, 2], F32, tag="d1iN")
            d2r = cp.tile([128, NITS, 2], F32, tag="d2r")
            d2i = cp.tile([128, NITS, 2], F32, tag="d2i")
            d2iN = cp.tile([128, NITS, 2], F32, tag="d2iN")
            zb = cp.tile([128, 1], F32, tag="zb")
            Wt = cp.tile([128, 6, 128], F32, tag="Wt")
            m0 = cp.tile([128, 2, 2, CPC], F32, tag="m0")
            sN = cp.tile([128, 2], F32, tag="sN")

            nc.sync.dma_start(
                th[:], thetas[:].rearrange("it (k e) -> k it e", k=128, e=2)
            )
            nc.sync.dma_start(m0[:], mask0[:])
            nc.sync.dma_start(Wt[:], wconst[:].rearrange("w p f -> p w f"))
            nc.vector.memset(zb[:], 0.0)

            wrp = cp.tile([128, 130, 2], F32, tag="wrp")
            nc.vector.tensor_scalar(wrp[:], th[:], PI, -2 * PI, mybir.AluOpType.is_gt, MULT)
            nc.vector.tensor_tensor(wrk[:], th[:], wrp[:], ADD)
            nc.scalar.activation(St[:], wrk[:], SIN, bias=zb[:])
            nc.vector.tensor_scalar(wrk[:], th[:], PI / 2, None, ADD)
            nc.vector.tensor_scalar(wrp[:], wrk[:], PI, -2 * PI, mybir.AluOpType.is_gt, MULT)
            nc.vector.tensor_tensor(wrk[:], wrk[:], wrp[:], ADD)
            nc.scalar.activation(Ct[:], wrk[:], SIN, bias=zb[:])

            Cmid = Ct[:, 1 : NITS + 1, :]
            Smid = St[:, 1 : NITS + 1, :]
            Csw = Ct[:, 1 : NITS + 1, ::-1]
            Ssw = St[:, 1 : NITS + 1, ::-1]
            wmid = wrk[:, :NITS, :]

            nc.vector.tensor_scalar(wmid, Csw, -AR * AR, None, MULT)
            nc.vector.scalar_tensor_tensor(d1r[:], Cmid, AT * AT, wmid, MULT, ADD)
            nc.vector.tensor_scalar(wmid, Ssw, -AR * AR, None, MULT)
            nc.vector.scalar_tensor_tensor(d1i[:], Smid, AT * AT, wmid, MULT, ADD)
            nc.vector.tensor_tensor(wmid, Smid, Ssw, ADD)
            nc.vector.tensor_scalar(d2r[:], wmid, -AT * AR, None, MULT)
            nc.vector.tensor_tensor(wmid, Cmid, Csw, ADD)
            nc.vector.tensor_scalar(d2i[:], wmid, AT * AR, None, MULT)

            nc.vector.tensor_scalar(d1iN[:], d1i[:], -1.0, None, MULT)
            nc.vector.tensor_scalar(d2iN[:], d2i[:], -1.0, None, MULT)
            nc.vector.tensor_scalar(sN[:], St[:, NITS + 1, :], -1.0, None, MULT)

            WdnG = Wt[:, 0, :]
            WdnNG = Wt[:, 1, :]
            WupG = Wt[:, 2, :]
            WupNG = Wt[:, 3, :]
            shift_w = {(0, 0): WdnNG, (0, 1): WdnG, (1, 0): WupNG, (1, 1): WupG}
            desc_w = (Wt[:, 4, :], Wt[:, 5, :])

            # ---- per-chain state ----
            chains = []
            for ci in range(2):
                st = dict(
                    X=sp.tile([128, 2, 2, HC], F32, tag=f"X{ci}", name=f"X{ci}"),
                    Y=sp.tile([128, 2, 2, HC], F32, tag=f"Y{ci}", name=f"Y{ci}"),
                    u=sp.tile([128, 2, 2, HC], F32, tag=f"u{ci}", name=f"u{ci}"),
                    t0=sp.tile([128, 2, HC], F32, tag=f"t0{ci}", name=f"t0{ci}"),
                    t1=sp.tile([128, 2, HC], F32, tag=f"t1{ci}", name=f"t1{ci}"),
                    P=None,
                    cols=slice(HC * ci, HC * (ci + 1)),
                    copy_eng=p["copy_engs"][ci],
                )
                chains.append(st)

            for ci, st in enumerate(chains):
                X = st["X"]
                cs = st["cols"]
                for e in range(2):
                    c0 = Ct[:, 0, e : e + 1]
                    s0 = St[:, 0, e : e + 1]
                    nc.vector.tensor_scalar(X[:, e, 0, :], m0[:, e, 0, cs], c0, None, MULT)
                    nc.vector.tensor_scalar(X[:, e, 1, :], m0[:, e, 1, cs], s0, None, MULT)

            def front(st, it):
                """copies + t/u phase for iteration it"""
                j = it - 1
                X, P, t0, t1, u = st["X"], st["P"], st["t0"], st["t1"], st["u"]
                cd2r = d2r[:, j, 0:1]
                cd2i = d2i[:, j, 0:1]
                cd2iN = d2iN[:, j, 0:1]
                if P is not None:
                    nc.vector.tensor_copy(X[:, :, 1, :], P[:, :, 1, :])
                    nc.vector.tensor_copy(X[:, :, 0, :], P[:, :, 0, :])
                XswR = X[:, ::-1, 0, :]
                XswI = X[:, ::-1, 1, :]
                if p["t0_eng"] == "act":
                    nc.scalar.activation(t0[:], XswI, COPY, scale=cd2iN)
                else:
                    _ENG[p["t0_eng"]](nc).tensor_scalar(t0[:], XswI, cd2iN, None, MULT)
                _ENG[p["t1_eng"]](nc).tensor_scalar(t1[:], XswI, cd2r, None, MULT)
                _ENG[p["u0_eng"]](nc).scalar_tensor_tensor(
                    u[:, :, 0, :], XswR, cd2r, t0[:], MULT, ADD
                )
                _ENG[p["u1_eng"]](nc).scalar_tensor_tensor(
                    u[:, :, 1, :], XswR, cd2i, t1[:], MULT, ADD
                )

            def back(st, it):
                """Y chains + CR matmuls for iteration it"""
                j = it - 1
                X, Y, u = st["X"], st["Y"], st["u"]
                cd1r = [d1r[:, j, e : e + 1] for e in range(2)]
                cd1i = [d1i[:, j, e : e + 1] for e in range(2)]
                cd1iN = [d1iN[:, j, e : e + 1] for e in range(2)]
                for e, pl, yeng in p["chains"]:
                    eng = _ENG[yeng](nc)
                    xa = X[:, e, 1 - pl, :]
                    xb = X[:, e, pl, :]
                    ca = cd1iN[e] if pl == 0 else cd1i[e]
                    eng.scalar_tensor_tensor(
                        Y[:, e, pl, :], xa, ca, u[:, e, pl, :], MULT, ADD
                    )
                    eng.scalar_tensor_tensor(
                        Y[:, e, pl, :], xb, cd1r[e], Y[:, e, pl, :], MULT, ADD
                    )
                if it == nits:
                    st["P"] = None
                    return
                P = pp.tile([128, 2, 2, HC], F32, tag="P", name="P")
                for e, pl in p["mm_order"]:
                    nc.tensor.matmul(
                        P[:, e, pl, :], shift_w[(e, pl)], Y[:, 1 - e, 1 - pl, :],
                        start=True, stop=False,
                    )
                    nc.tensor.matmul(
                        P[:, e, pl, :], desc_w[e], Y[:, e, pl, :],
                        start=False, stop=True,
                    )
                st["P"] = P

            # ---- phase-shifted main loop ----
            A, B = chains
            front(A, 1)
            for it in range(1, nits + 1):
                back(A, it)
                front(B, it)
                if it < nits:
                    front(A, it + 1)
                back(B, it)

            # ---- final: X = diag(p_129) @ Y ----
            for st in chains:
                X, Y, t0 = st["X"], st["Y"], st["t0"]
                for e in range(2):
                    c129 = Ct[:, NITS + 1, e : e + 1]
                    s129 = St[:, NITS + 1, e : e + 1]
                    s129N = sN[:, e : e + 1]
                    nc.vector.tensor_scalar(t0[:, e, :], Y[:, e, 1, :], s129N, None, MULT)
                    nc.vector.scalar_tensor_tensor(
                        X[:, e, 0, :], Y[:, e, 0, :], c129, t0[:, e, :], MULT, ADD
                    )
                    nc.vector.tensor_scalar(t0[:, e, :], Y[:, e, 0, :], s129, None, MULT)
                    nc.vector.scalar_tensor_tensor(
                        X[:, e, 1, :], Y[:, e, 1, :], c129, t0[:, e, :], MULT, ADD
                    )
                nc.sync.dma_start(out_d[:, :, :, st["cols"]], X[:])

    return nc


def build_nc3(nits=NITS, plan=None):
    """MMI-split row-layout chain.

    State S_it = MMI @ arch_{it-1} in row layout: one tile [128, T, pl, c]
    with partition = row mod 128, T = row tile (0: rows 0..127), pl = R/I.
    Recurrence (it = 1..127):  S_{it+1} = W @ (diag(p_it) * S_it)
    with W = MMI @ CR @ MMI a CONSTANT banded matrix -> pure PE weights.
    Then arch = diag(p129) * (MMI @ (diag(p128) * S_128)).

    Per step: 8 small diag ops (TS+STT per (T, pl)) + 16 const matmuls.

    wconst layout (lhsT tiles, see make_consts3):
      idx 0..3:  Wr[T][U]   (T,U) in (0,0),(0,1),(1,0),(1,1)
      idx 4..7:  Wi[T][U]
      idx 8..11: WiN[T][U]  (= -Wi)
      idx 12: at*I, idx 13: +ar*SW, idx 14: -ar*SW
    """
    p = dict(
        ts_engs=("dve", "dve"),                  # tm pass engine per T
        st_engs=("dve", "dve", "dve", "dve"),    # STT engine per (T, pl) slice
        slice_order=((0, 0), (1, 0), (0, 1), (1, 1)),
        mm_dt=mybir.dt.float16,                 # dtype for B tiles + W weights
        state_dt=mybir.dt.float16,
    )
    if plan:
        p.update(plan)
    MDT = p["mm_dt"]
    SDT = p["state_dt"]

    nc = bass.Bass()
    thetas = nc.dram_tensor("thetas", [130, N], F32, kind="ExternalInput")
    # init masks: [2(part), 2(T), 128, CPC]: at*1{r==col}, ar*1{r==partner}
    imask = nc.dram_tensor("imask", [2, 2, 128, CPC], F32, kind="ExternalInput")
    wconst = nc.dram_tensor("wconst", [27, 128, 128], MDT, kind="ExternalInput")
    out_d = nc.dram_tensor("out", [128, 2, 2, CPC], F32, kind="ExternalOutput")

    with tile.TileContext(nc) as tc:
        with (
            tc.tile_pool(name="state", bufs=1) as sp,
            tc.tile_pool(name="coef", bufs=1) as cp,
            tc.tile_pool(name="psum", bufs=2, space="PSUM") as pp,
        ):
            # ---- setup: row-layout trig (col 130 = pair-swapped theta_0) ----
            th = cp.tile([128, 2, 131], F32, tag="th")   # [r, T, it]
            Ct = cp.tile([128, 2, 131], F32, tag="Ct")
            St = cp.tile([128, 2, 131], F32, tag="St")
            wrk = cp.tile([128, 2, 131], F32, tag="wrk")
            wrp = cp.tile([128, 2, 131], F32, tag="wrp")
            zb = cp.tile([128, 1], F32, tag="zb")
            Wt = cp.tile([128, 27, 128], MDT, tag="Wt")
            mk = cp.tile([128, 2, 2, CPC], F32, tag="mk")

            nc.vector.memset(th[:], 0.0)
            for T in range(2):
                nc.sync.dma_start(
                    th[:, T, :130],
                    thetas[:, 128 * T : 128 * (T + 1)].rearrange("it r -> r it"),
                )
                thv = th[:, T, 130:131].rearrange("(k e) one -> k e one", e=2)
                nc.sync.dma_start(
                    thv[:, 0, :],
                    thetas[0:1, 128 * T + 1 : 128 * (T + 1) : 2].rearrange(
                        "one k -> k one"
                    ),
                )
                nc.sync.dma_start(
                    thv[:, 1, :],
                    thetas[0:1, 128 * T : 128 * (T + 1) : 2].rearrange(
                        "one k -> k one"
                    ),
                )
            nc.sync.dma_start(Wt[:], wconst[:].rearrange("w p f -> p w f"))
            nc.sync.dma_start(mk[:], imask[:].rearrange("m T p c -> p m T c"))
            nc.vector.memset(zb[:], 0.0)

            nc.vector.tensor_scalar(wrp[:], th[:], PI, -2 * PI, mybir.AluOpType.is_gt, MULT)
            nc.vector.tensor_tensor(wrk[:], th[:], wrp[:], ADD)
            nc.scalar.activation(St[:], wrk[:], SIN, bias=zb[:])
            nc.vector.tensor_scalar(wrk[:], th[:], PI / 2, None, ADD)
            nc.vector.tensor_scalar(wrp[:], wrk[:], PI, -2 * PI, mybir.AluOpType.is_gt, MULT)
            nc.vector.tensor_tensor(wrk[:], wrk[:], wrp[:], ADD)
            nc.scalar.activation(Ct[:], wrk[:], SIN, bias=zb[:])

            Wsets = []
            for si in range(2):
                Wr = {}
                Wi = {}
                WiN = {}
                for i, (T, U) in enumerate(((0, 0), (0, 1), (1, 0), (1, 1))):
                    Wr[(T, U)] = Wt[:, 12 * si + i, :]
                    Wi[(T, U)] = Wt[:, 12 * si + 4 + i, :]
                    WiN[(T, U)] = Wt[:, 12 * si + 8 + i, :]
                Wsets.append((Wr, Wi, WiN))
            M_at = Wt[:, 24, :]
            M_arP = Wt[:, 25, :]
            M_arN = Wt[:, 26, :]

            S = sp.tile([128, 2, 2, CPC], SDT, tag="S")   # [r, T, pl, c]
            B = sp.tile([128, 2, 2, CPC], MDT, tag="B")
            tm = sp.tile([128, 2, 2, CPC], SDT, tag="tm")
            Sf = sp.tile([128, 2, 2, CPC], F32, tag="Sf")
            tf = sp.tile([128, 2, 2, CPC], F32, tag="tf")
            Bf = sp.tile([128, 2, 2, CPC], F32, tag="Bf")

            # ---- init: S_1 = MMI @ diag(p0) restricted to this core's cols
            # S1_R = at*cos0*1{r=col} - ar*sin0*1{r=partner}
            # S1_I = at*sin0*1{r=col} + ar*cos0*1{r=partner}
            SUB = mybir.AluOpType.subtract
            for T in range(2):
                c0 = Ct[:, T, 0:1]
                s0 = St[:, T, 0:1]
                c0w = Ct[:, T, 130:131]  # trig of theta0[partner(r)]
                s0w = St[:, T, 130:131]
                mA = mk[:, 0, T, :]
                mB = mk[:, 1, T, :]
                nc.vector.tensor_scalar(tf[:, T, 0, :], mB, s0w, None, MULT)
                nc.vector.scalar_tensor_tensor(
                    S[:, T, 0, :], mA, c0, tf[:, T, 0, :], MULT, SUB
                )
                nc.vector.tensor_scalar(tf[:, T, 1, :], mB, c0w, None, MULT)
                nc.vector.scalar_tensor_tensor(
                    S[:, T, 1, :], mA, s0, tf[:, T, 1, :], MULT, ADD
                )

            def emit_diag(dst, src, it):
                """dst = diag(p_it) * src  (complex, per (T, pl) slices).
                tm[T,:] = sin_T * src[T, swapped-pl] in ONE op per T; then
                B[T,0] = cos*src[T,0] - tm[T,0], B[T,1] = cos*src[T,1] + tm[T,1].
                """
                for T, tse in zip((0, 1), p["ts_engs"]):
                    sin = St[:, T, it : it + 1]
                    if tse == "act":
                        nc.scalar.activation(
                            tm[:, T, :, :], src[:, T, ::-1, :], COPY, scale=sin
                        )
                    else:
                        _ENG[tse](nc).tensor_scalar(
                            tm[:, T, :, :], src[:, T, ::-1, :], sin, None, MULT
                        )
                for (T, pl), ste in zip(p["slice_order"], p["st_engs"]):
                    cos = Ct[:, T, it : it + 1]
                    _ENG[ste](nc).scalar_tensor_tensor(
                        dst[:, T, pl, :], src[:, T, pl, :], cos, tm[:, T, pl, :],
                        MULT, SUB if pl == 0 else ADD,
                    )

            # ---- main chain: 127 W-steps ----
            P = None
            for it in range(1, nits):
                if P is not None:
                    nc.vector.tensor_copy(S[:], P[:])
                emit_diag(B, S, it)
                Wr, Wi, WiN = Wsets[it % 2]
                P = pp.tile([128, 2, 2, CPC], F32, tag="P", name="P")
                for T in range(2):
                    U = 1 - T
                    # R out: Wr(TT)B[T,R] - Wi(TT)B[T,I] + corner equivalents
                    nc.tensor.matmul(P[:, T, 0, :], Wr[(T, T)], B[:, T, 0, :], start=True, stop=False)
                    nc.tensor.matmul(P[:, T, 0, :], WiN[(T, T)], B[:, T, 1, :], start=False, stop=False)
                    nc.tensor.matmul(P[:, T, 0, :], Wr[(T, U)], B[:, U, 0, :], start=False, stop=False)
                    nc.tensor.matmul(P[:, T, 0, :], WiN[(T, U)], B[:, U, 1, :], start=False, stop=True)
                    # I out: Wi(TT)B[T,R] + Wr(TT)B[T,I] + corners
                    nc.tensor.matmul(P[:, T, 1, :], Wi[(T, T)], B[:, T, 0, :], start=True, stop=False)
                    nc.tensor.matmul(P[:, T, 1, :], Wr[(T, T)], B[:, T, 1, :], start=False, stop=False)
                    nc.tensor.matmul(P[:, T, 1, :], Wi[(T, U)], B[:, U, 0, :], start=False, stop=False)
                    nc.tensor.matmul(P[:, T, 1, :], Wr[(T, U)], B[:, U, 1, :], start=False, stop=True)

            # ---- tail (fp32) ----
            nc.vector.tensor_copy(Sf[:], P[:])

            def tail_diag(dst, src, it):
                for T in range(2):
                    sin = St[:, T, it : it + 1]
                    cos = Ct[:, T, it : it + 1]
                    nc.vector.tensor_scalar(tf[:, T, :, :], src[:, T, ::-1, :], sin, None, MULT)
                    nc.vector.scalar_tensor_tensor(
                        dst[:, T, 0, :], src[:, T, 0, :], cos, tf[:, T, 0, :], MULT, SUB
                    )
                    nc.vector.scalar_tensor_tensor(
                        dst[:, T, 1, :], src[:, T, 1, :], cos, tf[:, T, 1, :], MULT, ADD
                    )

            tail_diag(Bf, Sf, nits)  # diag(p128) * S_128
            nc.vector.tensor_copy(B[:], Bf[:])  # fp16 moving for MMI matmuls
            PA = pp.tile([128, 2, 2, CPC], F32, tag="PA", name="PA")
            for T in range(2):
                # A_R = at*B_R - ar*SW*B_I ; A_I = at*B_I + ar*SW*B_R
                nc.tensor.matmul(PA[:, T, 0, :], M_at, B[:, T, 0, :], start=True, stop=False)
                nc.tensor.matmul(PA[:, T, 0, :], M_arN, B[:, T, 1, :], start=False, stop=True)
                nc.tensor.matmul(PA[:, T, 1, :], M_arP, B[:, T, 0, :], start=True, stop=False)
                nc.tensor.matmul(PA[:, T, 1, :], M_at, B[:, T, 1, :], start=False, stop=True)
            nc.vector.tensor_copy(Sf[:], PA[:])
            tail_diag(Bf, Sf, nits + 1)  # diag(p129) * A
            nc.sync.dma_start(out_d[:], Bf[:])

    return nc


def build_nc4(nits=NITS, plan=None):
    """Dual-chain MMI-split row-layout chain (see build_nc3).

    Two independent 16-column chains, phase-shifted half an iteration.
    Per chain-step: ACT copies PSUM->SBUF, Pool computes the two sin-cross
    passes (tensor_scalar), DVE the four cos +/- passes (STT), PE the 16
    fp16 const matmuls of W = MMI @ CR @ MMI.
    """
    p = dict(
        copy_engs=("dve", "dve"),      # full-tile PSUM->SBUF copy per chain
        ts_engs=("pool", "pool"),
        ts_psum=False,                 # tm reads PSUM directly (act/dve only)
        order="O1",                    # loop emission order
        mm_dt=mybir.dt.float16,
        st_engs=("dve", "dve", "dve", "dve"),
        slice_order=((0, 0), (1, 0), (0, 1), (1, 1)),
    )
    if plan:
        p.update(plan)
    MDT = p["mm_dt"]
    HC = CPC // 2
    SUB = mybir.AluOpType.subtract

    nc = bass.Bass()
    thetas = nc.dram_tensor("thetas", [130, N], F32, kind="ExternalInput")
    imask = nc.dram_tensor("imask", [2, 2, 128, CPC], F32, kind="ExternalInput")
    wconst = nc.dram_tensor("wconst", [15, 128, 128], MDT, kind="ExternalInput")
    out_d = nc.dram_tensor("out", [128, 2, 2, CPC], F32, kind="ExternalOutput")

    with tile.TileContext(nc) as tc:
        with (
            tc.tile_pool(name="state", bufs=1) as sp,
            tc.tile_pool(name="coef", bufs=1) as cp,
            tc.tile_pool(name="psum", bufs=4, space="PSUM") as pp,
        ):
            th = cp.tile([128, 2, 131], F32, tag="th")
            Ct = cp.tile([128, 2, 131], F32, tag="Ct")
            St = cp.tile([128, 2, 131], F32, tag="St")
            wrk = cp.tile([128, 2, 131], F32, tag="wrk")
            wrp = cp.tile([128, 2, 131], F32, tag="wrp")
            zb = cp.tile([128, 1], F32, tag="zb")
            Wt = cp.tile([128, 27, 128], MDT, tag="Wt")
            mk = cp.tile([128, 2, 2, CPC], F32, tag="mk")

            nc.vector.memset(th[:], 0.0)
            for T in range(2):
                nc.sync.dma_start(
                    th[:, T, :130],
                    thetas[:, 128 * T : 128 * (T + 1)].rearrange("it r -> r it"),
                )
                # column 130: pair-swapped theta_0 (theta0 of partner row),
                # needed by the init's partner-mask term; two strided DMAs
                # (even partitions <- odd cols, odd partitions <- even cols)
                thv = th[:, T, 130:131].rearrange("(k e) one -> k e one", e=2)
                nc.sync.dma_start(
                    thv[:, 0, :],
                    thetas[0:1, 128 * T + 1 : 128 * (T + 1) : 2].rearrange(
                        "one k -> k one"
                    ),
                )
                nc.sync.dma_start(
                    thv[:, 1, :],
                    thetas[0:1, 128 * T : 128 * (T + 1) : 2].rearrange(
                        "one k -> k one"
                    ),
                )
            nc.sync.dma_start(Wt[:], wconst[:].rearrange("w p f -> p w f"))
            nc.sync.dma_start(mk[:], imask[:].rearrange("m T p c -> p m T c"))
            nc.vector.memset(zb[:], 0.0)

            nc.vector.tensor_scalar(wrp[:], th[:], PI, -2 * PI, mybir.AluOpType.is_gt, MULT)
            nc.vector.tensor_tensor(wrk[:], th[:], wrp[:], ADD)
            nc.scalar.activation(St[:], wrk[:], SIN, bias=zb[:])
            nc.vector.tensor_scalar(wrk[:], th[:], PI / 2, None, ADD)
            nc.vector.tensor_scalar(wrp[:], wrk[:], PI, -2 * PI, mybir.AluOpType.is_gt, MULT)
            nc.vector.tensor_tensor(wrk[:], wrk[:], wrp[:], ADD)
            nc.scalar.activation(Ct[:], wrk[:], SIN, bias=zb[:])  # noqa

            Wr = {}
            Wi = {}
            WiN = {}
            for i, (T, U) in enumerate(((0, 0), (0, 1), (1, 0), (1, 1))):
                Wr[(T, U)] = Wt[:, i, :]
                Wi[(T, U)] = Wt[:, 4 + i, :]
                WiN[(T, U)] = Wt[:, 8 + i, :]
            M_at = Wt[:, 12, :]
            M_arP = Wt[:, 13, :]
            M_arN = Wt[:, 14, :]

            chains = []
            for ci in range(2):
                st = dict(
                    S=sp.tile([128, 2, 2, HC], F32, tag=f"S{ci}", name=f"S{ci}"),
                    B=sp.tile([128, 2, 2, HC], MDT, tag=f"B{ci}", name=f"B{ci}"),
                    tm=sp.tile([128, 2, 2, HC], F32, tag=f"tm{ci}", name=f"tm{ci}"),
                    P=None,
                    cols=slice(HC * ci, HC * (ci + 1)),
                    copy_eng=p["copy_engs"][ci],
                )
                chains.append(st)

            # init: S_1 = MMI @ diag(p0) columns
            for ci, st in enumerate(chains):
                S, tm, cs = st["S"], st["tm"], st["cols"]
                for T in range(2):
                    c0 = Ct[:, T, 0:1]
                    s0 = St[:, T, 0:1]
                    c0w = Ct[:, T, 130:131]  # cos(theta0[partner(r)])
                    s0w = St[:, T, 130:131]
                    mA = mk[:, 0, T, cs]
                    mB = mk[:, 1, T, cs]
                    nc.vector.tensor_scalar(tm[:, T, 0, :], mB, s0w, None, MULT)
                    nc.vector.scalar_tensor_tensor(
                        S[:, T, 0, :], mA, c0, tm[:, T, 0, :], MULT, SUB
                    )
                    nc.vector.tensor_scalar(tm[:, T, 1, :], mB, c0w, None, MULT)
                    nc.vector.scalar_tensor_tensor(
                        S[:, T, 1, :], mA, s0, tm[:, T, 1, :], MULT, ADD
                    )

            def front(st, it):
                """copy P->S, then tm = sin * S-swapped (per T)."""
                S, tm, P = st["S"], st["tm"], st["P"]
                if P is not None:
                    if st["copy_eng"] == "act":
                        nc.scalar.activation(S[:], P[:], COPY)
                    else:
                        nc.vector.tensor_copy(S[:], P[:])
                for T, tse in zip((0, 1), p["ts_engs"]):
                    sin = St[:, T, it : it + 1]
                    src = (
                        P[:, T, ::-1, :]
                        if (P is not None and p["ts_psum"])
                        else S[:, T, ::-1, :]
                    )
                    if tse == "act":
                        nc.scalar.activation(
                            tm[:, T, :, :], src, COPY, scale=sin
                        )
                    else:
                        _ENG[tse](nc).tensor_scalar(
                            tm[:, T, :, :], src, sin, None, MULT
                        )

            def back(st, it):
                """B = cos*S -/+ tm per slice; then the 16 W matmuls."""
                S, B, tm = st["S"], st["B"], st["tm"]
                for (T, pl), ste in zip(p["slice_order"], p["st_engs"]):
                    cos = Ct[:, T, it : it + 1]
                    _ENG[ste](nc).scalar_tensor_tensor(
                        B[:, T, pl, :], S[:, T, pl, :], cos, tm[:, T, pl, :],
                        MULT, SUB if pl == 0 else ADD,
                    )
                P = pp.tile([128, 2, 2, HC], F32, tag="P", name="P")
                for T in range(2):
                    U = 1 - T
                    nc.tensor.matmul(P[:, T, 0, :], Wr[(T, T)], B[:, T, 0, :], start=True, stop=False)
                    nc.tensor.matmul(P[:, T, 0, :], WiN[(T, T)], B[:, T, 1, :], start=False, stop=False)
                    nc.tensor.matmul(P[:, T, 0, :], Wr[(T, U)], B[:, U, 0, :], start=False, stop=False)
                    nc.tensor.matmul(P[:, T, 0, :], WiN[(T, U)], B[:, U, 1, :], start=False, stop=True)
                    nc.tensor.matmul(P[:, T, 1, :], Wi[(T, T)], B[:, T, 0, :], start=True, stop=False)
                    nc.tensor.matmul(P[:, T, 1, :], Wr[(T, T)], B[:, T, 1, :], start=False, stop=False)
                    nc.tensor.matmul(P[:, T, 1, :], Wi[(T, U)], B[:, U, 0, :], start=False, stop=False)
                    nc.tensor.matmul(P[:, T, 1, :], Wr[(T, U)], B[:, U, 1, :], start=False, stop=True)
                st["P"] = P

            # main loop over W-steps it = 1..127, phase-shifted chains
            A, Bc = chains
            front(A, 1)
            for it in range(1, nits):
                if p["order"] == "O1":
                    back(A, it)
                    front(Bc, it)
                    if it < nits - 1:
                        front(A, it + 1)
                    back(Bc, it)
                else:  # O2
                    back(A, it)
                    front(Bc, it)
                    back(Bc, it)
                    if it < nits - 1:
                        front(A, it + 1)

            # tail per chain: P holds S_128; A = MMI @ (diag(p128)*S128);
            # out = diag(p129) * A.
            for st in chains:
                S, B, tm, P = st["S"], st["B"], st["tm"], st["P"]
                if P is not None:
                    nc.scalar.activation(S[:, :, 0, :], P[:, :, 0, :], COPY)
                    nc.scalar.activation(S[:, :, 1, :], P[:, :, 1, :], COPY)
                # B = diag(p128) * S128
                for T in range(2):
                    sin = St[:, T, nits : nits + 1]
                    cos = Ct[:, T, nits : nits + 1]
                    nc.vector.tensor_scalar(tm[:, T, :, :], S[:, T, ::-1, :], sin, None, MULT)
                    nc.vector.scalar_tensor_tensor(
                        B[:, T, 0, :], S[:, T, 0, :], cos, tm[:, T, 0, :], MULT, SUB
                    )
                    nc.vector.scalar_tensor_tensor(
                        B[:, T, 1, :], S[:, T, 1, :], cos, tm[:, T, 1, :], MULT, ADD
                    )
                PA = pp.tile([128, 2, 2, HC], F32, tag="PA", name="PA")
                for T in range(2):
                    nc.tensor.matmul(PA[:, T, 0, :], M_at, B[:, T, 0, :], start=True, stop=False)
                    nc.tensor.matmul(PA[:, T, 0, :], M_arN, B[:, T, 1, :], start=False, stop=True)
                    nc.tensor.matmul(PA[:, T, 1, :], M_arP, B[:, T, 0, :], start=True, stop=False)
                    nc.tensor.matmul(PA[:, T, 1, :], M_at, B[:, T, 1, :], start=False, stop=True)
                nc.vector.tensor_copy(S[:], PA[:])
                Bo = sp.tile([128, 2, 2, HC], F32, tag="Bo", name="Bo")
                for T in range(2):
                    sin = St[:, T, nits + 1 : nits + 2]
                    cos = Ct[:, T, nits + 1 : nits + 2]
                    nc.vector.tensor_scalar(tm[:, T, :, :], S[:, T, ::-1, :], sin, None, MULT)
                    nc.vector.scalar_tensor_tensor(
                        Bo[:, T, 0, :], S[:, T, 0, :], cos, tm[:, T, 0, :], MULT, SUB
                    )
                    nc.vector.scalar_tensor_tensor(
                        Bo[:, T, 1, :], S[:, T, 1, :], cos, tm[:, T, 1, :], MULT, ADD
                    )
                nc.sync.dma_start(out_d[:, :, :, st["cols"]], Bo[:])

    return nc


def build_nc5(nits=NITS, plan=None):
    """Single-chain MMI-split with PE-negated I plane + fp16 state.

    PSUM P holds [128, T, (R, I, Ineg), c] where Ineg = -I comes from extra
    matmuls with negated weights.  Then per T the whole diag multiply is:
        tm[T,:] = sin_T * P-copy[T, (Ineg, R)]   (one TS, both planes)
        B[T,:]  = cos_T * S[T, (R, I)] + tm[T,:] (one STT, uniform ADD)
    State S, tm, B all fp16 (DVE 2x); weights alternate between two
    complement-rounded fp16 sets so rounding bias cancels across steps.

    wconst layout (lhsT): [0:16) set-a, [16:32) set-b, each ordered
    Wr(4 blocks), Wi(4), WiN(4), WrN(4); then at*I, +ar*SW, -ar*SW.
    """
    p = dict(
        ts_engs=("dve", "dve"),
        st_engs=("dve", "dve"),
        mm_dt=mybir.dt.float16,
        state_dt=mybir.dt.float16,
    )
    if plan:
        p.update(plan)
    MDT = p["mm_dt"]
    SDT = p["state_dt"]
    SUB = mybir.AluOpType.subtract

    nc = bass.Bass()
    thetas = nc.dram_tensor("thetas", [130, N], F32, kind="ExternalInput")
    imask = nc.dram_tensor("imask", [2, 2, 128, CPC], F32, kind="ExternalInput")
    wconst = nc.dram_tensor("wconst", [35, 128, 128], MDT, kind="ExternalInput")
    out_d = nc.dram_tensor("out", [128, 2, 2, CPC], F32, kind="ExternalOutput")

    with tile.TileContext(nc) as tc:
        with (
            tc.tile_pool(name="state", bufs=1) as sp,
            tc.tile_pool(name="coef", bufs=1) as cp,
            tc.tile_pool(name="psum", bufs=2, space="PSUM") as pp,
        ):
            th = cp.tile([128, 2, 131], F32, tag="th")
            Ct = cp.tile([128, 2, 131], F32, tag="Ct")
            St = cp.tile([128, 2, 131], F32, tag="St")
            wrk = cp.tile([128, 2, 131], F32, tag="wrk")
            wrp = cp.tile([128, 2, 131], F32, tag="wrp")
            zb = cp.tile([128, 1], F32, tag="zb")
            Wt = cp.tile([128, 35, 128], MDT, tag="Wt")
            mk = cp.tile([128, 2, 2, CPC], F32, tag="mk")

            nc.vector.memset(th[:], 0.0)
            for T in range(2):
                nc.sync.dma_start(
                    th[:, T, :130],
                    thetas[:, 128 * T : 128 * (T + 1)].rearrange("it r -> r it"),
                )
                thv = th[:, T, 130:131].rearrange("(k e) one -> k e one", e=2)
                nc.sync.dma_start(
                    thv[:, 0, :],
                    thetas[0:1, 128 * T + 1 : 128 * (T + 1) : 2].rearrange(
                        "one k -> k one"
                    ),
                )
                nc.sync.dma_start(
                    thv[:, 1, :],
                    thetas[0:1, 128 * T : 128 * (T + 1) : 2].rearrange(
                        "one k -> k one"
                    ),
                )
            nc.sync.dma_start(Wt[:], wconst[:].rearrange("w p f -> p w f"))
            nc.sync.dma_start(mk[:], imask[:].rearrange("m T p c -> p m T c"))
            nc.vector.memset(zb[:], 0.0)

            nc.vector.tensor_scalar(wrp[:], th[:], PI, -2 * PI, mybir.AluOpType.is_gt, MULT)
            nc.vector.tensor_tensor(wrk[:], th[:], wrp[:], ADD)
            nc.scalar.activation(St[:], wrk[:], SIN, bias=zb[:])
            nc.vector.tensor_scalar(wrk[:], th[:], PI / 2, None, ADD)
            nc.vector.tensor_scalar(wrp[:], wrk[:], PI, -2 * PI, mybir.AluOpType.is_gt, MULT)
            nc.vector.tensor_tensor(wrk[:], wrk[:], wrp[:], ADD)
            nc.scalar.activation(Ct[:], wrk[:], SIN, bias=zb[:])

            Wsets = []
            for si in range(2):
                base = 16 * si
                Wr = {}
                Wi = {}
                WiN = {}
                WrN = {}
                for i, (T, U) in enumerate(((0, 0), (0, 1), (1, 0), (1, 1))):
                    Wr[(T, U)] = Wt[:, base + i, :]
                    Wi[(T, U)] = Wt[:, base + 4 + i, :]
                    WiN[(T, U)] = Wt[:, base + 8 + i, :]
                    WrN[(T, U)] = Wt[:, base + 12 + i, :]
                Wsets.append((Wr, Wi, WiN, WrN))
            M_at = Wt[:, 32, :]
            M_arP = Wt[:, 33, :]
            M_arN = Wt[:, 34, :]

            S = sp.tile([128, 2, 3, CPC], SDT, tag="S")   # [r, T, (R,I,In), c]
            B = sp.tile([128, 2, 2, CPC], SDT, tag="B")
            tm = sp.tile([128, 2, 2, CPC], SDT, tag="tm")
            Sf = sp.tile([128, 2, 2, CPC], F32, tag="Sf")  # fp32 tail state
            tf = sp.tile([128, 2, 2, CPC], F32, tag="tf")

            # ---- init: S_1 = MMI @ diag(p0) cols (fp16 state + Ineg plane)
            for T in range(2):
                c0 = Ct[:, T, 0:1]
                s0 = St[:, T, 0:1]
                c0w = Ct[:, T, 130:131]
                s0w = St[:, T, 130:131]
                mA = mk[:, 0, T, :]
                mB = mk[:, 1, T, :]
                nc.vector.tensor_scalar(tf[:, T, 0, :], mB, s0w, None, MULT)
                nc.vector.scalar_tensor_tensor(
                    S[:, T, 0, :], mA, c0, tf[:, T, 0, :], MULT, SUB
                )
                nc.vector.tensor_scalar(tf[:, T, 1, :], mB, c0w, None, MULT)
                nc.vector.scalar_tensor_tensor(
                    S[:, T, 1, :], mA, s0, tf[:, T, 1, :], MULT, ADD
                )
                nc.vector.tensor_scalar(S[:, T, 2, :], S[:, T, 1, :], -1.0, None, MULT)

            # ---- main chain: 127 W-steps ----
            P = None
            for it in range(1, nits):
                if P is not None:
                    nc.vector.tensor_copy(S[:], P[:])
                Wr, Wi, WiN, WrN = Wsets[it % 2]
                for T, tse in zip((0, 1), p["ts_engs"]):
                    sin = St[:, T, it : it + 1]
                    # tm[T, 0] = sin * S[T, Ineg]; tm[T, 1] = sin * S[T, R]
                    _ENG[tse](nc).tensor_scalar(
                        tm[:, T, :, :], S[:, T, 2::-2, :], sin, None, MULT
                    )
                for T, ste in zip((0, 1), p["st_engs"]):
                    cos = Ct[:, T, it : it + 1]
                    _ENG[ste](nc).scalar_tensor_tensor(
                        B[:, T, :, :], S[:, T, 0:2, :], cos, tm[:, T, :, :],
                        MULT, ADD,
                    )
                P = pp.tile([128, 2, 3, CPC], F32, tag="P", name="P")
                for T in range(2):
                    U = 1 - T
                    nc.tensor.matmul(P[:, T, 0, :], Wr[(T, T)], B[:, T, 0, :], start=True, stop=False)
                    nc.tensor.matmul(P[:, T, 0, :], WiN[(T, T)], B[:, T, 1, :], start=False, stop=False)
                    nc.tensor.matmul(P[:, T, 0, :], Wr[(T, U)], B[:, U, 0, :], start=False, stop=False)
                    nc.tensor.matmul(P[:, T, 0, :], WiN[(T, U)], B[:, U, 1, :], start=False, stop=True)
                    nc.tensor.matmul(P[:, T, 1, :], Wi[(T, T)], B[:, T, 0, :], start=True, stop=False)
                    nc.tensor.matmul(P[:, T, 1, :], Wr[(T, T)], B[:, T, 1, :], start=False, stop=False)
                    nc.tensor.matmul(P[:, T, 1, :], Wi[(T, U)], B[:, U, 0, :], start=False, stop=False)
                    nc.tensor.matmul(P[:, T, 1, :], Wr[(T, U)], B[:, U, 1, :], start=False, stop=True)
                    nc.tensor.matmul(P[:, T, 2, :], WiN[(T, T)], B[:, T, 0, :], start=True, stop=False)
                    nc.tensor.matmul(P[:, T, 2, :], WrN[(T, T)], B[:, T, 1, :], start=False, stop=False)
                    nc.tensor.matmul(P[:, T, 2, :], WiN[(T, U)], B[:, U, 0, :], start=False, stop=False)
                    nc.tensor.matmul(P[:, T, 2, :], WrN[(T, U)], B[:, U, 1, :], start=False, stop=True)

            # ---- tail (fp32): S128 from P planes (R, I); diag(p128); MMI; diag(p129)
            nc.vector.tensor_copy(Sf[:], P[:, :, 0:2, :])
            Bf = sp.tile([128, 2, 2, CPC], F32, tag="Bf")
            for T in range(2):
                sin = St[:, T, nits : nits + 1]
                cos = Ct[:, T, nits : nits + 1]
                nc.vector.tensor_scalar(tf[:, T, :, :], Sf[:, T, ::-1, :], sin, None, MULT)
                nc.vector.scalar_tensor_tensor(
                    Bf[:, T, 0, :], Sf[:, T, 0, :], cos, tf[:, T, 0, :], MULT, SUB
                )
                nc.vector.scalar_tensor_tensor(
                    Bf[:, T, 1, :], Sf[:, T, 1, :], cos, tf[:, T, 1, :], MULT, ADD
                )
            # MMI matmuls need fp16 moving operand: round Bf -> B (one copy)
            nc.vector.tensor_copy(B[:], Bf[:])
            PA = pp.tile([128, 2, 2, CPC], F32, tag="PA", name="PA")
            for T in range(2):
                nc.tensor.matmul(PA[:, T, 0, :], M_at, B[:, T, 0, :], start=True, stop=False)
                nc.tensor.matmul(PA[:, T, 0, :], M_arN, B[:, T, 1, :], start=False, stop=True)
                nc.tensor.matmul(PA[:, T, 1, :], M_arP, B[:, T, 0, :], start=True, stop=False)
                nc.tensor.matmul(PA[:, T, 1, :], M_at, B[:, T, 1, :], start=False, stop=True)
            nc.vector.tensor_copy(Sf[:], PA[:])
            for T in range(2):
                sin = St[:, T, nits + 1 : nits + 2]
                cos = Ct[:, T, nits + 1 : nits + 2]
                nc.vector.tensor_scalar(tf[:, T, :, :], Sf[:, T, ::-1, :], sin, None, MULT)
                nc.vector.scalar_tensor_tensor(
                    Bf[:, T, 0, :], Sf[:, T, 0, :], cos, tf[:, T, 0, :], MULT, SUB
                )
                nc.vector.scalar_tensor_tensor(
                    Bf[:, T, 1, :], Sf[:, T, 1, :], cos, tf[:, T, 1, :], MULT, ADD
                )
            nc.sync.dma_start(out_d[:], Bf[:])

    return nc


def make_consts5():
    """wconst for build_nc5: two complement-rounded fp16 W sets + MMI tiles."""
    a = np.sqrt(1.0 - IL_MMI)
    t = np.sqrt((1.0 + IMB) / 2.0)
    r = np.sqrt((1.0 - IMB) / 2.0)
    mmi2 = a * np.array([[t, 1j * r], [1j * r, t]], dtype=np.complex64)
    MMI = np.kron(np.eye(N // 2, dtype=np.complex64), mmi2)
    acr = np.sqrt(1.0 - IL_CR)
    thru = acr * np.sqrt(1.0 - CT)
    cr2 = acr * np.array(
        [[np.sqrt(CT), 1j * np.sqrt(1.0 - CT)],
         [1j * np.sqrt(1.0 - CT), np.sqrt(CT)]], dtype=np.complex64
    )
    CR = np.zeros((N, N), dtype=np.complex64)
    CR[0, 0] = thru
    for i in range(1, N - 1, 2):
        CR[i : i + 2, i : i + 2] = cr2
    CR[-1, -1] = thru
    W = (MMI @ CR @ MMI).astype(np.complex64)

    blocks = [(0, 0), (0, 1), (1, 0), (1, 1)]

    def wtiles(Wc):
        out = []
        for part in ("r", "i", "iN", "rN"):
            for T, U in blocks:
                blk = Wc[128 * T : 128 * (T + 1), 128 * U : 128 * (U + 1)]
                v = {"r": blk.real, "i": blk.imag, "iN": -blk.imag, "rN": -blk.real}[part]
                out.append(np.ascontiguousarray(v.T).astype(np.float32))
        return out

    seta32 = wtiles(W)
    seta = [x.astype(np.float16) for x in seta32]
    # complement set: fp16(2W - fp16(W)) so that (a + b)/2 ~= W exactly-ish
    setb = [
        (2.0 * x32 - x16.astype(np.float32)).astype(np.float16)
        for x32, x16 in zip(seta32, seta)
    ]
    at = a * t
    ar = a * r
    SW = np.kron(np.eye(64, dtype=np.float32), np.array([[0, 1], [1, 0]], np.float32))
    tail = [
        (at * np.eye(128, dtype=np.float32)).astype(np.float16),
        (ar * SW).astype(np.float16),
        (-ar * SW).astype(np.float16),
    ]
    return np.stack(seta + setb + tail)


def make_consts6():
    """wconst for updated build_nc3: two complement-rounded fp16 W sets
    (12 tiles each: Wr x4, Wi x4, WiN x4) + 3 MMI tail tiles."""
    a = np.sqrt(1.0 - IL_MMI)
    t = np.sqrt((1.0 + IMB) / 2.0)
    r = np.sqrt((1.0 - IMB) / 2.0)
    mmi2 = a * np.array([[t, 1j * r], [1j * r, t]], dtype=np.complex64)
    MMI = np.kron(np.eye(N // 2, dtype=np.complex64), mmi2)
    acr = np.sqrt(1.0 - IL_CR)
    thru = acr * np.sqrt(1.0 - CT)
    cr2 = acr * np.array(
        [[np.sqrt(CT), 1j * np.sqrt(1.0 - CT)],
         [1j * np.sqrt(1.0 - CT), np.sqrt(CT)]], dtype=np.complex64
    )
    CR = np.zeros((N, N), dtype=np.complex64)
    CR[0, 0] = thru
    for i in range(1, N - 1, 2):
        CR[i : i + 2, i : i + 2] = cr2
    CR[-1, -1] = thru
    W = (MMI @ CR @ MMI).astype(np.complex64)

    blocks = [(0, 0), (0, 1), (1, 0), (1, 1)]
    tiles32 = []
    for part in ("r", "i", "iN"):
        for T, U in blocks:
            blk = W[128 * T : 128 * (T + 1), 128 * U : 128 * (U + 1)]
            v = {"r": blk.real, "i": blk.imag, "iN": -blk.imag}[part]
            tiles32.append(np.ascontiguousarray(v.T).astype(np.float32))
    seta = [x.astype(np.float16) for x in tiles32]
    setb = [
        (2.0 * x32 - x16.astype(np.float32)).astype(np.float16)
        for x32, x16 in zip(tiles32, seta)
    ]
    at = a * t
    ar = a * r
    SW = np.kron(np.eye(64, dtype=np.float32), np.array([[0, 1], [1, 0]], np.float32))
    tail = [
        (at * np.eye(128, dtype=np.float32)).astype(np.float16),
        (ar * SW).astype(np.float16),
        (-ar * SW).astype(np.float16),
    ]
    return np.stack(seta + setb + tail)


def make_consts3():
    """Host constants for build_nc3: W = MMI @ CR @ MMI blocks (lhsT) etc."""
    a = np.sqrt(1.0 - IL_MMI)
    t = np.sqrt((1.0 + IMB) / 2.0)
    r = np.sqrt((1.0 - IMB) / 2.0)
    mmi2 = a * np.array([[t, 1j * r], [1j * r, t]], dtype=np.complex64)
    MMI = np.kron(np.eye(N // 2, dtype=np.complex64), mmi2)
    acr = np.sqrt(1.0 - IL_CR)
    thru = acr * np.sqrt(1.0 - CT)
    cr2 = acr * np.array(
        [[np.sqrt(CT), 1j * np.sqrt(1.0 - CT)],
         [1j * np.sqrt(1.0 - CT), np.sqrt(CT)]], dtype=np.complex64
    )
    CR = np.zeros((N, N), dtype=np.complex64)
    CR[0, 0] = thru
    for i in range(1, N - 1, 2):
        CR[i : i + 2, i : i + 2] = cr2
    CR[-1, -1] = thru
    W = (MMI @ CR @ MMI).astype(np.complex64)

    tiles = []
    blocks = [(0, 0), (0, 1), (1, 0), (1, 1)]
    for T, U in blocks:
        blk = W[128 * T : 128 * (T + 1), 128 * U : 128 * (U + 1)]
        tiles.append(np.ascontiguousarray(blk.real.T))
    for T, U in blocks:
        blk = W[128 * T : 128 * (T + 1), 128 * U : 128 * (U + 1)]
        tiles.append(np.ascontiguousarray(blk.imag.T))
    for T, U in blocks:
        blk = W[128 * T : 128 * (T + 1), 128 * U : 128 * (U + 1)]
        tiles.append(np.ascontiguousarray(-blk.imag.T))
    at = a * t
    ar = a * r
    SW = np.kron(np.eye(64, dtype=np.float32), np.array([[0, 1], [1, 0]], np.float32))
    tiles.append(at * np.eye(128, dtype=np.float32))
    tiles.append(ar * SW)   # symmetric, lhsT = itself
    tiles.append(-ar * SW)
    return np.stack(tiles).astype(np.float32)


def make_imask3(core: int) -> np.ndarray:
    """imask[m, T, r, c]: m=0: at*1{row==col}, m=1: ar*1{row==partner(col)}."""
    a = np.sqrt(1.0 - IL_MMI)
    t = np.sqrt((1.0 + IMB) / 2.0)
    r_ = np.sqrt((1.0 - IMB) / 2.0)
    at = a * t
    ar = a * r_
    out = np.zeros((2, 2, 128, CPC), np.float32)
    for c in range(CPC):
        col = CPC * core + c
        par = col ^ 1  # pair partner (cols pair as (2k, 2k+1))
        out[0, col // 128, col % 128, c] = at
        out[1, par // 128, par % 128, c] = ar
    return out


def make_consts():
    """Constant weights: g2-scaled shift permutations + diag(g1) (lhsT form)."""
    wdn = np.eye(128, k=1, dtype=np.float32)   # lhsT[p,f] = (f == p+1)
    wup = np.eye(128, k=-1, dtype=np.float32)  # lhsT[p,f] = (f == p-1)
    g = np.full((128, 2), G1S, dtype=np.float32)
    g[0, 0] = G2C
    g[127, 1] = G2C
    wconst = np.stack(
        [
            G2C * wdn,
            -G2C * wdn,
            G2C * wup,
            -G2C * wup,
            np.diag(g[:, 0]),
            np.diag(g[:, 1]),
        ]
    ).astype(np.float32)
    return wconst


def make_mask0(core: int) -> np.ndarray:
    """mask0[k,e,l,c] = 1 iff global row 2k+e == global col 32*core+c."""
    k = np.arange(128)[:, None, None, None]
    e = np.arange(2)[None, :, None, None]
    c = np.arange(CPC)[None, None, None, :]
    m = (2 * k + e == CPC * core + c).astype(np.float32)
    return np.broadcast_to(m, (128, 2, 2, CPC)).copy()


_CACHE = {}


def _get_nc():
    if "nc" not in _CACHE:
        nc = build_nc()
        fix_sync_waits(nc)
        _CACHE["nc"] = nc
    return _CACHE["nc"]


def _run(thetas: np.ndarray, trace: bool = False):
    thetas = np.ascontiguousarray(thetas, dtype=np.float32)
    assert thetas.shape == (130, N)
    nc = _get_nc()
    wconst = make_consts()
    in_maps = [
        {"thetas": thetas, "mask0": make_mask0(c), "wconst": wconst}
        for c in range(NCORES)
    ]
    res = run_bass_kernel_spmd(nc, in_maps, list(range(NCORES)), trace=trace)
    out = np.empty((N, N), dtype=np.complex64)
    for c in range(NCORES):
        o = res.results[c]["out"]  # [128, 2, 2, CPC]
        blk = o[:, :, 0, :] + 1j * o[:, :, 1, :]  # [128, 2, CPC]
        out[:, CPC * c : CPC * (c + 1)] = blk.reshape(N, CPC)
    return out, res


def _run3(thetas: np.ndarray, trace: bool = False):
    thetas = np.ascontiguousarray(thetas, dtype=np.float32)
    assert thetas.shape == (130, N)
    if "nc3" not in _CACHE:
        nc = build_nc3()
        fix_sync_waits(nc)
        _CACHE["nc3"] = nc
    nc = _CACHE["nc3"]
    wconst = make_consts3().astype(np.float16)
    in_maps = [
        {"thetas": thetas, "imask": make_imask3(c), "wconst": wconst}
        for c in range(NCORES)
    ]
    res = run_bass_kernel_spmd(nc, in_maps, list(range(NCORES)), trace=trace)
    out = np.empty((N, N), dtype=np.complex64)
    for c in range(NCORES):
        o = res.results[c]["out"]  # [128(r), 2(T), 2(pl), CPC]
        blk = o[:, :, 0, :] + 1j * o[:, :, 1, :]  # [r, T, c]
        full = blk.transpose(1, 0, 2).reshape(N, CPC)
        out[:, CPC * c : CPC * (c + 1)] = full
    return out, res


BEST_PLAN4 = dict(copy_engs=("dve", "dve"))
BEST_PLAN3 = dict()


def _run3v2(thetas: np.ndarray, trace: bool = False):
    thetas = np.ascontiguousarray(thetas, dtype=np.float32)
    assert thetas.shape == (130, N)
    if "nc3v2" not in _CACHE:
        nc = build_nc3(plan=BEST_PLAN3)
        fix_sync_waits(nc)
        _CACHE["nc3v2"] = nc
    nc = _CACHE["nc3v2"]
    wconst = make_consts6().astype(np.float16)
    in_maps = [
        {"thetas": thetas, "imask": make_imask3(c), "wconst": wconst}
        for c in range(NCORES)
    ]
    res = run_bass_kernel_spmd(nc, in_maps, list(range(NCORES)), trace=trace)
    out = np.empty((N, N), dtype=np.complex64)
    for c in range(NCORES):
        o = res.results[c]["out"]
        blk = o[:, :, 0, :] + 1j * o[:, :, 1, :]
        out[:, CPC * c : CPC * (c + 1)] = blk.transpose(1, 0, 2).reshape(N, CPC)
    return out, res


def _run4(thetas: np.ndarray, trace: bool = False):
    thetas = np.ascontiguousarray(thetas, dtype=np.float32)
    assert thetas.shape == (130, N)
    if "nc4" not in _CACHE:
        nc = build_nc4(plan=BEST_PLAN4)
        fix_sync_waits(nc)
        _CACHE["nc4"] = nc
    nc = _CACHE["nc4"]
    wconst = make_consts3().astype(np.float16)
    in_maps = [
        {"thetas": thetas, "imask": make_imask3(c), "wconst": wconst}
        for c in range(NCORES)
    ]
    res = run_bass_kernel_spmd(nc, in_maps, list(range(NCORES)), trace=trace)
    out = np.empty((N, N), dtype=np.complex64)
    for c in range(NCORES):
        o = res.results[c]["out"]  # [128(r), 2(T), 2(pl), CPC]
        blk = o[:, :, 0, :] + 1j * o[:, :, 1, :]  # [r, T, c]
        full = blk.transpose(1, 0, 2).reshape(N, CPC)
        out[:, CPC * c : CPC * (c + 1)] = full
    return out, res


def build_active():
    """The nc variant kernel() currently uses (for test harness timing)."""
    nc = build_nc3(plan=BEST_PLAN3)
    return nc


def _run_active(thetas, trace=False):
    return _run3v2(thetas, trace=trace)


def kernel(thetas: np.ndarray) -> np.ndarray:
    out, _ = _run3v2(thetas, trace=False)
    return out
